# revision 25
# baseline (speedup 1.0000x reference)
"""Trainium2 Bass kernel for nn_DropLearner2 (IIR filter bank + MLP edge gating).

Two-stage design: scores kernel -> host index routing -> gating kernel.

The lfilter along the feature axis is a linear operator y = x @ M with
M = (L_a^{-1} L_b)^T computed on the host from the 12 filter coefficients,
so the whole IIR scan folds into the MLP's first-layer matmul weights.
Stage 1 computes all 4 (branch, band) combo scores for each core's node
slice on the PE (bf16 operands, fp32 PSUM accumulation).  The u_add_v
per-edge index routing runs on the host (this stack's per-element
indirect-DMA lowering is broken: walrus emits row-granular indirection
only and vector_dynamic_offsets NEFFs fail to load).  Stage 2 evaluates
the concrete-relaxation gate per edge on ACT/DVE with per-core partial
sums for the scalar reg.
"""

import numpy as np

import concourse.bacc as bacc
import concourse.mybir as mybir
import concourse.tile as tile
from concourse import bass_utils

AF = mybir.ActivationFunctionType
ALU = mybir.AluOpType
AX = mybir.AxisListType
F32 = mybir.dt.float32
BF16 = mybir.dt.bfloat16

TEMPERATURE = 0.5
BIAS = 0.0001


class Cfg2:
    def __init__(self, N=50000, D=256, H=64, E=1600000, ncores=8,
                 n_hi=None, xdtype="bf16"):
        self.N, self.D, self.H, self.E, self.ncores = N, D, H, E, ncores
        self.n_hi = int(0.2 * N) if n_hi is None else n_hi
        self.n_lo = N - self.n_hi
        self.NLOC = N // ncores
        # lo region: all nodes; hi region: only nodes needing a hi-band
        # score (~36% of nodes; generous static bound + host assert)
        self.NLOCP = -(-self.NLOC // 128) * 128
        frac = 1.0 - (self.n_lo / N) ** 2   # P(src-hi or dst-hi)
        nhi_exp = self.NLOC * frac
        self.NHIP = max(128, int(-(-(nhi_exp + 5 * nhi_exp ** 0.5) // 128))
                        * 128)
        self.TOT = self.NLOCP + self.NHIP
        self.tiles = []   # (col offset, width, region 0=lo 1=hi)
        off = 0
        while off < self.NLOCP:
            w = min(512, self.NLOCP - off)
            self.tiles.append((off, w, 0))
            off += w
        while off < self.TOT:
            w = min(512, self.TOT - off)
            self.tiles.append((off, w, 1))
            off += w
        self.P = 128
        self.EC = -(-E // ncores)
        self.FW = -(-self.EC // self.P)
        self.SLOT = self.P * self.FW
        self.xdtype = xdtype


def _lfilter_matrix(b, a, D):
    """M such that lfilter(b, a, X) == X @ M for row-wise X (f64)."""
    b = np.asarray(b, np.float64)
    a = np.asarray(a, np.float64)
    b = b / a[0]
    a = a / a[0]
    La = np.zeros((D, D))
    Lb = np.zeros((D, D))
    for k in range(len(a)):
        i = np.arange(k, D)
        La[i, i - k] = a[k]
        Lb[i, i - k] = b[k]
    return np.linalg.solve(La, Lb).T


def build_nc1(cfg):
    """Stage 1: 4-combo score strip per core."""
    c = cfg
    XD = BF16 if c.xdtype == "bf16" else F32
    nc = bacc.Bacc("TRN2", target_bir_lowering=False, debug=False,
                   num_devices=c.ncores)
    xT_d = nc.dram_tensor("xT", [c.D, c.TOT], XD, kind="ExternalInput")
    # wcat packed [128, 512]: 128-col blocks (dk, rg) at col (2*dk+rg)*128
    wcat_d = nc.dram_tensor("wcat", [128, 512], XD, kind="ExternalInput")
    b1_d = nc.dram_tensor("b1cat", [128, 1], F32, kind="ExternalInput")
    w2_d = nc.dram_tensor("w2cat", [128, 2], XD, kind="ExternalInput")
    sc_d = nc.dram_tensor("scores", [2, c.TOT], F32,
                          kind="ExternalOutput")

    with tile.TileContext(nc) as tc:
        with (
            tc.tile_pool(name="const", bufs=1) as constp,
            tc.tile_pool(name="xin", bufs=4) as xin,
            tc.tile_pool(name="hbuf", bufs=3) as hbuf,
            tc.tile_pool(name="pbig", bufs=2, space="PSUM") as pbig,
            tc.tile_pool(name="psml", bufs=2, space="PSUM") as psml,
        ):
            wall = constp.tile([128, 512], XD, tag="wall")
            nc.scalar.dma_start(out=wall[:], in_=wcat_d[:, :])
            wt = {(dk, rg): wall[:, (2 * dk + rg) * 128:
                                 (2 * dk + rg + 1) * 128]
                  for dk in range(2) for rg in range(2)}
            b1t = constp.tile([128, 1], F32, tag="b1")
            nc.scalar.dma_start(out=b1t[:], in_=b1_d[:, :])
            w2t = constp.tile([128, 2], XD, tag="w2")
            nc.scalar.dma_start(out=w2t[:], in_=w2_d[:, :])

            scores = constp.tile([2, c.TOT], F32, tag="scores")
            # group node tiles into quarters with a ramp-up (small first
            # group so the first matmul starts early); one DMA per group
            sizes = [1, 2, 4]
            qgroups = []
            i = 0
            for sz in sizes:
                if i >= len(c.tiles):
                    break
                qgroups.append(c.tiles[i:i + sz])
                i += sz
            while i < len(c.tiles):
                qgroups.append(c.tiles[i:i + 5])
                i += 5
            qtiles = []
            for qi, grp in enumerate(qgroups):
                q0 = grp[0][0]
                qw = grp[-1][0] + grp[-1][1] - q0
                qt = xin.tile([128, 2 * qw], XD, tag=f"xq{qi}")
                # layout: [:, 0:qw] = xT rows 0:128, [:, qw:] = rows 128:256
                nc.sync.dma_start(
                    out=qt[:].rearrange("p (k w) -> p k w", k=2),
                    in_=xT_d[:, q0:q0 + qw].rearrange("(k p) w -> p k w",
                                                      k=2))
                qtiles.append((qt, q0, qw))
            ti = 0
            for qi, grp in enumerate(qgroups):
                qt, q0, qw = qtiles[qi]
                for (lo, w, rg) in grp:
                    o = lo - q0
                    ph = ti % 2          # alternate psum/h buffers
                    ps = pbig.tile([128, 512], F32, tag=f"p{ph}")
                    nc.tensor.matmul(ps[:, :w], wt[0, rg],
                                     qt[:, o:o + w], start=True, stop=False)
                    nc.tensor.matmul(ps[:, :w], wt[1, rg],
                                     qt[:, qw + o:qw + o + w],
                                     start=False, stop=True)
                    h = hbuf.tile([128, 512], XD, tag=f"h{ph}")
                    nc.scalar.activation(h[:, :w], ps[:, :w], AF.Relu,
                                         bias=b1t[:, 0:1], scale=1.0)
                    pss = psml.tile([2, 512], F32, tag=f"s{ph}")
                    nc.tensor.matmul(pss[:, :w], w2t[:, 0:2],
                                     h[:, :w], start=True, stop=True)
                    nc.vector.tensor_copy(scores[0:2, lo:lo + w],
                                          pss[:, :w])
                    ti += 1
                # ship each quarter's scores as they complete
                nc.sync.dma_start(out=sc_d[:, q0:q0 + qw],
                                  in_=scores[:, q0:q0 + qw])
    nc.compile()
    return nc


def build_nc2(cfg):
    """Stage 2: per-edge gating."""
    c = cfg
    nc = bacc.Bacc("TRN2", target_bir_lowering=False, debug=False,
                   num_devices=c.ncores)
    ue_d = nc.dram_tensor("ueps", [c.P, c.FW], F32, kind="ExternalInput")
    w_d = nc.dram_tensor("wsum", [c.P, c.FW], F32, kind="ExternalInput")
    aug_d = nc.dram_tensor("aug", [c.P, c.FW], F32, kind="ExternalOutput")
    regp_d = nc.dram_tensor("regp", [c.P, 4], F32, kind="ExternalOutput")

    # two equal chunks (bigger ACT/DVE ops won over finer pipelining)
    FC = -(-c.FW // 2)
    chunks = [(0, FC), (FC, c.FW - FC)]
    NCH = len(chunks)

    with tile.TileContext(nc) as tc:
        with (
            tc.tile_pool(name="const", bufs=1) as constp,
            tc.tile_pool(name="ech", bufs=3) as ech,
        ):
            biast = constp.tile([128, 2], F32, tag="biast")
            nc.vector.memset(biast[:, 0:1], 1.0 - BIAS)
            nc.vector.memset(biast[:, 1:2], BIAS)
            racc = constp.tile([c.P, 4], F32, tag="racc")

            # phase A: logit(eps) + w per chunk (ACT table: Ln once);
            # chunked input DMAs so compute starts after the first chunk
            t1s = []
            for ch, (lo, w) in enumerate(chunks):
                hi = lo + w
                ue = ech.tile([c.P, FC], F32, tag="ue")
                nc.sync.dma_start(out=ue[:, :w], in_=ue_d[:, lo:hi])
                ws = ech.tile([c.P, FC], F32, tag="ws")
                # wsum streams on the scalar HWDGE queue in parallel
                nc.scalar.dma_start(out=ws[:, :w], in_=w_d[:, lo:hi])
                lp = ech.tile([c.P, FC], F32, tag="lp")
                nc.scalar.activation(lp[:, :w], ue[:, :w], AF.Ln,
                                     bias=biast[:, 0:1],
                                     scale=-(1.0 - 2 * BIAS))
                lq = ech.tile([c.P, FC], F32, tag="lq")
                nc.scalar.activation(lq[:, :w], ue[:, :w], AF.Ln,
                                     bias=biast[:, 1:2],
                                     scale=(1.0 - 2 * BIAS))
                t0 = ech.tile([c.P, FC], F32, tag="t0")
                nc.vector.tensor_tensor(t0[:, :w], lp[:, :w], lq[:, :w],
                                        ALU.subtract)
                t1 = constp.tile([c.P, FC], F32, tag=f"t1_{ch}")
                nc.vector.tensor_tensor(t1[:, :w], t0[:, :w], ws[:, :w],
                                        ALU.add)
                t1s.append((t1, lo, w))

            # phase B: sigmoid with fused per-partition row sums
            # (accum_out); ship the raw [P, NCH] accumulators as regp
            # (host sums them) so no reduce/PE tail remains
            for ch, (t1, lo, w) in enumerate(t1s):
                au = ech.tile([c.P, FC], F32, tag="au")
                nc.scalar.activation(au[:, :w], t1[:, :w], AF.Sigmoid,
                                     bias=0.0, scale=1.0 / TEMPERATURE,
                                     accum_out=racc[:, ch:ch + 1])
                half = w // 2
                nc.sync.dma_start(out=aug_d[:, lo:lo + half],
                                  in_=au[:, :half])
                nc.sync.dma_start(out=aug_d[:, lo + half:lo + w],
                                  in_=au[:, half:w])
            nc.sync.dma_start(out=regp_d[:, :], in_=racc[:])
    nc.compile()
    return nc


_CACHE = {}


def _get(key, fn):
    if key not in _CACHE:
        _CACHE[key] = fn()
    return _CACHE[key]


def host_weights(cfg, inputs):
    c = cfg
    M_lo = _lfilter_matrix(inputs["b_lo"], inputs["a_lo"], c.D)
    M_hi = _lfilter_matrix(inputs["b_hi"], inputs["a_hi"], c.D)
    w1s = np.asarray(inputs["w1_src"], np.float64)
    w1d = np.asarray(inputs["w1_dst"], np.float64)
    # [D, 256]: cols 0:128 lo band [w1_src | w1_dst], 128:256 hi band;
    # then packed to [128, 512] with block (dk, rg) at col (2*dk+rg)*128
    wc = np.concatenate(
        [M_lo @ w1s, M_lo @ w1d, M_hi @ w1s, M_hi @ w1d], axis=1
    ).astype(np.float32)
    wcat = np.concatenate(
        [wc[dk * 128:(dk + 1) * 128, rg * 128:(rg + 1) * 128]
         for dk in range(2) for rg in range(2)], axis=1)
    b1cat = np.concatenate(
        [inputs["b1_src"], inputs["b1_dst"]]).astype(np.float32).reshape(
            128, 1)
    w2cat = np.zeros((128, 2), np.float32)
    w2cat[:c.H, 0] = np.asarray(inputs["w2_src"]).ravel()
    w2cat[c.H:, 1] = np.asarray(inputs["w2_dst"]).ravel()
    b2sum = float(np.asarray(inputs["b2_src"]).ravel()[0]
                  + np.asarray(inputs["b2_dst"]).ravel()[0])
    return wcat, b1cat, w2cat, b2sum


def run2(cfg, inputs, trace=False):
    c = cfg
    xdt = mybir.dt.np(BF16) if c.xdtype == "bf16" else np.float32
    wcat, b1cat, w2cat, b2sum = host_weights(c, inputs)
    xT = np.asarray(inputs["node_emb"], np.float32).T

    # hi-group: nodes whose hi-band score is referenced by any position
    need_hi = np.zeros(c.N, bool)
    need_hi[np.asarray(inputs["idx_src_hi"], np.int64)] = True
    need_hi[np.asarray(inputs["idx_dst_hi"], np.int64)] = True
    # hicol[n]: column of node n's hi-band scores within its core's strip
    hicol = np.full(c.N, -1, np.int64)
    in_maps1 = []
    for k in range(c.ncores):
        n0 = k * c.NLOC
        sl = np.zeros((c.D, c.TOT), np.float32)
        sl[:, :c.NLOC] = xT[:, n0:n0 + c.NLOC]
        hn = n0 + np.flatnonzero(need_hi[n0:n0 + c.NLOC])
        assert len(hn) <= c.NHIP, (len(hn), c.NHIP)
        sl[:, c.NLOCP:c.NLOCP + len(hn)] = xT[:, hn]
        hicol[hn] = c.NLOCP + np.arange(len(hn))
        in_maps1.append({"xT": sl.astype(xdt), "wcat": wcat.astype(xdt),
                         "b1cat": b1cat, "w2cat": w2cat.astype(xdt)})

    nc1 = _get(("nc1fd", c.N, c.ncores, c.xdtype), lambda: build_nc1(c))
    res1 = bass_utils.run_bass_kernel_spmd(
        nc1, in_maps1, core_ids=list(range(c.ncores)), trace=trace)

    # per-core strips [2, TOT] -> flat table.  strip row 0 = src-branch
    # scores, row 1 = dst-branch; cols 0:NLOCP lo band, NLOCP: hi band.
    table = np.concatenate(
        [res1.results[k]["scores"].ravel() for k in range(c.ncores)])

    def pos_lo(node, row):
        return ((node // c.NLOC) * 2 * c.TOT + row * c.TOT
                + node % c.NLOC)

    def pos_hi(node, row):
        return (node // c.NLOC) * 2 * c.TOT + row * c.TOT + hicol[node]

    pos_src = np.empty(c.N, np.int64)
    pos_dst = np.empty(c.N, np.int64)
    pos_src[:c.n_lo] = pos_lo(np.asarray(inputs["idx_src_lo"], np.int64), 0)
    pos_src[c.n_lo:] = pos_hi(np.asarray(inputs["idx_src_hi"], np.int64), 0)
    pos_dst[:c.n_lo] = pos_lo(np.asarray(inputs["idx_dst_lo"], np.int64), 1)
    pos_dst[c.n_lo:] = pos_hi(np.asarray(inputs["idx_dst_hi"], np.int64), 1)

    src = np.asarray(inputs["src"], np.int64)
    dst = np.asarray(inputs["dst"], np.int64)
    wsum = (table[pos_src[src]] + table[pos_dst[dst]]
            + np.float32(b2sum)).astype(np.float32)

    u_eps = np.asarray(inputs["u_eps"], np.float32)
    in_maps2 = []
    for k in range(c.ncores):
        s, e = k * c.SLOT, min((k + 1) * c.SLOT, c.E)
        n = e - s
        ue = np.full(c.SLOT, 0.5, np.float32)
        ws = np.full(c.SLOT, -1e4, np.float32)   # pads -> sigmoid == 0
        if n > 0:
            ue[:n] = u_eps[s:e]
            ws[:n] = wsum[s:e]
        in_maps2.append({"ueps": ue.reshape(c.P, c.FW),
                         "wsum": ws.reshape(c.P, c.FW)})

    nc2 = _get(("nc2fb", c.E, c.ncores), lambda: build_nc2(c))
    res2 = bass_utils.run_bass_kernel_spmd(
        nc2, in_maps2, core_ids=list(range(c.ncores)), trace=trace)

    aug_full = np.empty(c.E, np.float32)
    reg_sum = 0.0
    for k in range(c.ncores):
        s, e = k * c.SLOT, min((k + 1) * c.SLOT, c.E)
        aug_full[s:e] = res2.results[k]["aug"].ravel()[:e - s]
        reg_sum += float(res2.results[k]["regp"].sum())
    reg = np.float32(1.0 - reg_sum / c.E)
    return (reg, aug_full[:, None, None]), (res1, res2)


def kernel(**inputs):
    cfg = Cfg2()
    (reg, aug), _ = run2(cfg, inputs)
    return reg, aug


# revision 26
# speedup vs baseline: 1.0422x; 1.0422x over previous
"""Trainium2 Bass kernel for nn_DropLearner2 (IIR filter bank + MLP edge gating).

Two-stage design: scores kernel -> host index routing -> gating kernel.

The lfilter along the feature axis is a linear operator y = x @ M with
M = (L_a^{-1} L_b)^T computed on the host from the 12 filter coefficients,
so the whole IIR scan folds into the MLP's first-layer matmul weights.
Stage 1 computes all 4 (branch, band) combo scores for each core's node
slice on the PE (bf16 operands, fp32 PSUM accumulation).  The u_add_v
per-edge index routing runs on the host (this stack's per-element
indirect-DMA lowering is broken: walrus emits row-granular indirection
only and vector_dynamic_offsets NEFFs fail to load).  Stage 2 evaluates
the concrete-relaxation gate per edge on ACT/DVE with per-core partial
sums for the scalar reg.
"""

import numpy as np

import concourse.bacc as bacc
import concourse.mybir as mybir
import concourse.tile as tile
from concourse import bass_utils

AF = mybir.ActivationFunctionType
ALU = mybir.AluOpType
AX = mybir.AxisListType
F32 = mybir.dt.float32
BF16 = mybir.dt.bfloat16

TEMPERATURE = 0.5
BIAS = 0.0001


class Cfg2:
    def __init__(self, N=50000, D=256, H=64, E=1600000, ncores=8,
                 n_hi=None, xdtype="bf16"):
        self.N, self.D, self.H, self.E, self.ncores = N, D, H, E, ncores
        self.n_hi = int(0.2 * N) if n_hi is None else n_hi
        self.n_lo = N - self.n_hi
        self.NLOC = N // ncores
        # lo region: all nodes; hi region: only nodes needing a hi-band
        # score (~36% of nodes; generous static bound + host assert)
        self.NLOCP = -(-self.NLOC // 128) * 128
        frac = 1.0 - (self.n_lo / N) ** 2   # P(src-hi or dst-hi)
        nhi_exp = self.NLOC * frac
        self.NHIP = max(128, int(-(-(nhi_exp + 5 * nhi_exp ** 0.5) // 128))
                        * 128)
        self.TOT = self.NLOCP + self.NHIP
        self.tiles = []   # (col offset, width, region 0=lo 1=hi)
        off = 0
        while off < self.NLOCP:
            w = min(512, self.NLOCP - off)
            self.tiles.append((off, w, 0))
            off += w
        while off < self.TOT:
            w = min(512, self.TOT - off)
            self.tiles.append((off, w, 1))
            off += w
        self.P = 128
        self.EC = -(-E // ncores)
        self.FW = -(-self.EC // self.P)
        self.SLOT = self.P * self.FW
        self.xdtype = xdtype


def _lfilter_matrix(b, a, D):
    """M such that lfilter(b, a, X) == X @ M for row-wise X (f64)."""
    b = np.asarray(b, np.float64)
    a = np.asarray(a, np.float64)
    b = b / a[0]
    a = a / a[0]
    La = np.zeros((D, D))
    Lb = np.zeros((D, D))
    for k in range(len(a)):
        i = np.arange(k, D)
        La[i, i - k] = a[k]
        Lb[i, i - k] = b[k]
    return np.linalg.solve(La, Lb).T


def build_nc1(cfg):
    """Stage 1: 4-combo score strip per core."""
    c = cfg
    XD = BF16 if c.xdtype == "bf16" else F32
    nc = bacc.Bacc("TRN2", target_bir_lowering=False, debug=False,
                   num_devices=c.ncores)
    xT_d = nc.dram_tensor("xT", [c.D, c.TOT], XD, kind="ExternalInput")
    # wcat packed [128, 512]: 128-col blocks (dk, rg) at col (2*dk+rg)*128
    wcat_d = nc.dram_tensor("wcat", [128, 512], XD, kind="ExternalInput")
    b1_d = nc.dram_tensor("b1cat", [128, 1], F32, kind="ExternalInput")
    w2_d = nc.dram_tensor("w2cat", [128, 2], XD, kind="ExternalInput")
    sc_d = nc.dram_tensor("scores", [2, c.TOT], F32,
                          kind="ExternalOutput")

    with tile.TileContext(nc) as tc:
        with (
            tc.tile_pool(name="const", bufs=1) as constp,
            tc.tile_pool(name="xin", bufs=4) as xin,
            tc.tile_pool(name="hbuf", bufs=3) as hbuf,
            tc.tile_pool(name="pbig", bufs=2, space="PSUM") as pbig,
            tc.tile_pool(name="psml", bufs=2, space="PSUM") as psml,
        ):
            wall = constp.tile([128, 512], XD, tag="wall")
            nc.scalar.dma_start(out=wall[:], in_=wcat_d[:, :])
            wt = {(dk, rg): wall[:, (2 * dk + rg) * 128:
                                 (2 * dk + rg + 1) * 128]
                  for dk in range(2) for rg in range(2)}
            b1t = constp.tile([128, 1], F32, tag="b1")
            nc.scalar.dma_start(out=b1t[:], in_=b1_d[:, :])
            w2t = constp.tile([128, 2], XD, tag="w2")
            nc.scalar.dma_start(out=w2t[:], in_=w2_d[:, :])

            scores = constp.tile([2, c.TOT], F32, tag="scores")
            # group node tiles into quarters with a ramp-up (small first
            # group so the first matmul starts early); one DMA per group
            sizes = [1, 2, 4]
            qgroups = []
            i = 0
            for sz in sizes:
                if i >= len(c.tiles):
                    break
                qgroups.append(c.tiles[i:i + sz])
                i += sz
            while i < len(c.tiles):
                qgroups.append(c.tiles[i:i + 5])
                i += 5
            qtiles = []
            for qi, grp in enumerate(qgroups):
                q0 = grp[0][0]
                qw = grp[-1][0] + grp[-1][1] - q0
                qt = xin.tile([128, 2 * qw], XD, tag=f"xq{qi}")
                # layout: [:, 0:qw] = xT rows 0:128, [:, qw:] = rows 128:256
                nc.sync.dma_start(
                    out=qt[:].rearrange("p (k w) -> p k w", k=2),
                    in_=xT_d[:, q0:q0 + qw].rearrange("(k p) w -> p k w",
                                                      k=2))
                qtiles.append((qt, q0, qw))
            ti = 0
            for qi, grp in enumerate(qgroups):
                qt, q0, qw = qtiles[qi]
                for (lo, w, rg) in grp:
                    o = lo - q0
                    ph = ti % 2          # alternate psum/h buffers
                    ps = pbig.tile([128, 512], F32, tag=f"p{ph}")
                    nc.tensor.matmul(ps[:, :w], wt[0, rg],
                                     qt[:, o:o + w], start=True, stop=False)
                    nc.tensor.matmul(ps[:, :w], wt[1, rg],
                                     qt[:, qw + o:qw + o + w],
                                     start=False, stop=True)
                    h = hbuf.tile([128, 512], XD, tag=f"h{ph}")
                    nc.scalar.activation(h[:, :w], ps[:, :w], AF.Relu,
                                         bias=b1t[:, 0:1], scale=1.0)
                    pss = psml.tile([2, 512], F32, tag=f"s{ph}")
                    nc.tensor.matmul(pss[:, :w], w2t[:, 0:2],
                                     h[:, :w], start=True, stop=True)
                    nc.vector.tensor_copy(scores[0:2, lo:lo + w],
                                          pss[:, :w])
                    ti += 1
                # ship each quarter's scores as they complete
                nc.sync.dma_start(out=sc_d[:, q0:q0 + qw],
                                  in_=scores[:, q0:q0 + qw])
    nc.compile()
    return nc


def build_nc2(cfg):
    """Stage 2: per-edge gating."""
    c = cfg
    nc = bacc.Bacc("TRN2", target_bir_lowering=False, debug=False,
                   num_devices=c.ncores)
    ue_d = nc.dram_tensor("ueps", [c.P, c.FW], F32, kind="ExternalInput")
    w_d = nc.dram_tensor("wsum", [c.P, c.FW], F32, kind="ExternalInput")
    aug_d = nc.dram_tensor("aug", [c.P, c.FW], F32, kind="ExternalOutput")
    regp_d = nc.dram_tensor("regp", [c.P, 4], F32, kind="ExternalOutput")

    # two equal chunks (bigger ACT/DVE ops won over finer pipelining)
    FC = -(-c.FW // 2)
    chunks = [(0, FC), (FC, c.FW - FC)]
    NCH = len(chunks)

    with tile.TileContext(nc) as tc:
        with (
            tc.tile_pool(name="const", bufs=1) as constp,
            tc.tile_pool(name="ech", bufs=3) as ech,
        ):
            biast = constp.tile([128, 2], F32, tag="biast")
            nc.vector.memset(biast[:, 0:1], 1.0 - BIAS)
            nc.vector.memset(biast[:, 1:2], BIAS)
            racc = constp.tile([c.P, 4], F32, tag="racc")

            # phase A: logit(eps) + w per chunk (ACT table: Ln once);
            # chunked input DMAs so compute starts after the first chunk
            t1s = []
            for ch, (lo, w) in enumerate(chunks):
                hi = lo + w
                ue = ech.tile([c.P, FC], F32, tag="ue")
                nc.sync.dma_start(out=ue[:, :w], in_=ue_d[:, lo:hi])
                ws = ech.tile([c.P, FC], F32, tag="ws")
                nc.sync.dma_start(out=ws[:, :w], in_=w_d[:, lo:hi])
                lp = ech.tile([c.P, FC], F32, tag="lp")
                nc.scalar.activation(lp[:, :w], ue[:, :w], AF.Ln,
                                     bias=biast[:, 0:1],
                                     scale=-(1.0 - 2 * BIAS))
                lq = ech.tile([c.P, FC], F32, tag="lq")
                nc.scalar.activation(lq[:, :w], ue[:, :w], AF.Ln,
                                     bias=biast[:, 1:2],
                                     scale=(1.0 - 2 * BIAS))
                t0 = ech.tile([c.P, FC], F32, tag="t0")
                nc.vector.tensor_tensor(t0[:, :w], lp[:, :w], lq[:, :w],
                                        ALU.subtract)
                t1 = constp.tile([c.P, FC], F32, tag=f"t1_{ch}")
                nc.vector.tensor_tensor(t1[:, :w], t0[:, :w], ws[:, :w],
                                        ALU.add)
                t1s.append((t1, lo, w))

            # phase B: sigmoid with fused per-partition row sums
            # (accum_out); ship the raw [P, NCH] accumulators as regp
            # (host sums them) so no reduce/PE tail remains
            for ch, (t1, lo, w) in enumerate(t1s):
                au = ech.tile([c.P, FC], F32, tag="au")
                nc.scalar.activation(au[:, :w], t1[:, :w], AF.Sigmoid,
                                     bias=0.0, scale=1.0 / TEMPERATURE,
                                     accum_out=racc[:, ch:ch + 1])
                half = w // 2
                nc.sync.dma_start(out=aug_d[:, lo:lo + half],
                                  in_=au[:, :half])
                nc.sync.dma_start(out=aug_d[:, lo + half:lo + w],
                                  in_=au[:, half:w])
            nc.sync.dma_start(out=regp_d[:, :], in_=racc[:])
    nc.compile()
    return nc


_CACHE = {}


def _get(key, fn):
    if key not in _CACHE:
        _CACHE[key] = fn()
    return _CACHE[key]


def host_weights(cfg, inputs):
    c = cfg
    M_lo = _lfilter_matrix(inputs["b_lo"], inputs["a_lo"], c.D)
    M_hi = _lfilter_matrix(inputs["b_hi"], inputs["a_hi"], c.D)
    w1s = np.asarray(inputs["w1_src"], np.float64)
    w1d = np.asarray(inputs["w1_dst"], np.float64)
    # [D, 256]: cols 0:128 lo band [w1_src | w1_dst], 128:256 hi band;
    # then packed to [128, 512] with block (dk, rg) at col (2*dk+rg)*128
    wc = np.concatenate(
        [M_lo @ w1s, M_lo @ w1d, M_hi @ w1s, M_hi @ w1d], axis=1
    ).astype(np.float32)
    wcat = np.concatenate(
        [wc[dk * 128:(dk + 1) * 128, rg * 128:(rg + 1) * 128]
         for dk in range(2) for rg in range(2)], axis=1)
    b1cat = np.concatenate(
        [inputs["b1_src"], inputs["b1_dst"]]).astype(np.float32).reshape(
            128, 1)
    w2cat = np.zeros((128, 2), np.float32)
    w2cat[:c.H, 0] = np.asarray(inputs["w2_src"]).ravel()
    w2cat[c.H:, 1] = np.asarray(inputs["w2_dst"]).ravel()
    b2sum = float(np.asarray(inputs["b2_src"]).ravel()[0]
                  + np.asarray(inputs["b2_dst"]).ravel()[0])
    return wcat, b1cat, w2cat, b2sum


def run2(cfg, inputs, trace=False):
    c = cfg
    xdt = mybir.dt.np(BF16) if c.xdtype == "bf16" else np.float32
    wcat, b1cat, w2cat, b2sum = host_weights(c, inputs)
    xT = np.asarray(inputs["node_emb"], np.float32).T

    # hi-group: nodes whose hi-band score is referenced by any position
    need_hi = np.zeros(c.N, bool)
    need_hi[np.asarray(inputs["idx_src_hi"], np.int64)] = True
    need_hi[np.asarray(inputs["idx_dst_hi"], np.int64)] = True
    # hicol[n]: column of node n's hi-band scores within its core's strip
    hicol = np.full(c.N, -1, np.int64)
    in_maps1 = []
    for k in range(c.ncores):
        n0 = k * c.NLOC
        sl = np.zeros((c.D, c.TOT), np.float32)
        sl[:, :c.NLOC] = xT[:, n0:n0 + c.NLOC]
        hn = n0 + np.flatnonzero(need_hi[n0:n0 + c.NLOC])
        assert len(hn) <= c.NHIP, (len(hn), c.NHIP)
        sl[:, c.NLOCP:c.NLOCP + len(hn)] = xT[:, hn]
        hicol[hn] = c.NLOCP + np.arange(len(hn))
        in_maps1.append({"xT": sl.astype(xdt), "wcat": wcat.astype(xdt),
                         "b1cat": b1cat, "w2cat": w2cat.astype(xdt)})

    nc1 = _get(("nc1fd", c.N, c.ncores, c.xdtype), lambda: build_nc1(c))
    res1 = bass_utils.run_bass_kernel_spmd(
        nc1, in_maps1, core_ids=list(range(c.ncores)), trace=trace)

    # per-core strips [2, TOT] -> flat table.  strip row 0 = src-branch
    # scores, row 1 = dst-branch; cols 0:NLOCP lo band, NLOCP: hi band.
    table = np.concatenate(
        [res1.results[k]["scores"].ravel() for k in range(c.ncores)])

    def pos_lo(node, row):
        return ((node // c.NLOC) * 2 * c.TOT + row * c.TOT
                + node % c.NLOC)

    def pos_hi(node, row):
        return (node // c.NLOC) * 2 * c.TOT + row * c.TOT + hicol[node]

    pos_src = np.empty(c.N, np.int64)
    pos_dst = np.empty(c.N, np.int64)
    pos_src[:c.n_lo] = pos_lo(np.asarray(inputs["idx_src_lo"], np.int64), 0)
    pos_src[c.n_lo:] = pos_hi(np.asarray(inputs["idx_src_hi"], np.int64), 0)
    pos_dst[:c.n_lo] = pos_lo(np.asarray(inputs["idx_dst_lo"], np.int64), 1)
    pos_dst[c.n_lo:] = pos_hi(np.asarray(inputs["idx_dst_hi"], np.int64), 1)

    src = np.asarray(inputs["src"], np.int64)
    dst = np.asarray(inputs["dst"], np.int64)
    wsum = (table[pos_src[src]] + table[pos_dst[dst]]
            + np.float32(b2sum)).astype(np.float32)

    u_eps = np.asarray(inputs["u_eps"], np.float32)
    in_maps2 = []
    for k in range(c.ncores):
        s, e = k * c.SLOT, min((k + 1) * c.SLOT, c.E)
        n = e - s
        ue = np.full(c.SLOT, 0.5, np.float32)
        ws = np.full(c.SLOT, -1e4, np.float32)   # pads -> sigmoid == 0
        if n > 0:
            ue[:n] = u_eps[s:e]
            ws[:n] = wsum[s:e]
        in_maps2.append({"ueps": ue.reshape(c.P, c.FW),
                         "wsum": ws.reshape(c.P, c.FW)})

    nc2 = _get(("nc2eb", c.E, c.ncores), lambda: build_nc2(c))
    res2 = bass_utils.run_bass_kernel_spmd(
        nc2, in_maps2, core_ids=list(range(c.ncores)), trace=trace)

    aug_full = np.empty(c.E, np.float32)
    reg_sum = 0.0
    for k in range(c.ncores):
        s, e = k * c.SLOT, min((k + 1) * c.SLOT, c.E)
        aug_full[s:e] = res2.results[k]["aug"].ravel()[:e - s]
        reg_sum += float(res2.results[k]["regp"].sum())
    reg = np.float32(1.0 - reg_sum / c.E)
    return (reg, aug_full[:, None, None]), (res1, res2)


def kernel(**inputs):
    cfg = Cfg2()
    (reg, aug), _ = run2(cfg, inputs)
    return reg, aug
